# revision 1
# baseline (speedup 1.0000x reference)
"""Single-head causal attention on 8 TRN2 NeuronCores, batch-parallel.

Per core (1 batch element): x [2048,1024] f32, Wq/Wk/Wv [1024,64] f32.
  qkT = [Wq|Wk]^T @ x^T   (fused projection, f32r matmuls)
  ST[s,t] = k_s . q_t     (scores computed transposed, K=64)
  P = exp(ST/8), causal-masked via 0/1 mask tiles (no max-subtraction:
      inputs are bounded, |S| < ~7, exp cannot overflow)
  OT[h,t] = sum_s v'[s,h] P[s,t] with v' = [v | 1] so row 64 = softmax
      denominator; final O = (OT[:64]/OT[64]).T
"""
import numpy as np

import concourse.bass as bass
import concourse.mybir as mybir
import concourse.tile as tile
from concourse import bacc, bass_utils
from concourse.masks import make_identity

P = 128
T = 2048
C = 1024
H = 64
NT = T // P      # 16 t-blocks
NCC = C // P     # 8 c-chunks
F32 = mybir.dt.float32
F32R = mybir.dt.float32r
BF16 = mybir.dt.bfloat16
SCALE = 1.0 / np.sqrt(H)

_CACHE = {}


def build_program(trace_friendly=False, niter=1):
    nc = bacc.Bacc("TRN2", target_bir_lowering=False, debug=False, num_devices=8)
    # x declared f32r: same 4-byte storage; PE transposes run at 1.5 vs 2
    # cycles/row and feed f32r projections directly.
    x_d = nc.dram_tensor("x", [T, C], F32R, kind="ExternalInput").ap()
    wq_d = nc.dram_tensor("Wq", [C, H], F32, kind="ExternalInput").ap()
    wk_d = nc.dram_tensor("Wk", [C, H], F32, kind="ExternalInput").ap()
    wv_d = nc.dram_tensor("Wv", [C, H], F32, kind="ExternalInput").ap()
    o_d = nc.dram_tensor("out", [T, H], F32, kind="ExternalOutput").ap()

    body = _body_v2
    with tile.TileContext(nc) as tc:
        if niter == 1:
            body(nc, tc, x_d, wq_d, wk_d, wv_d, o_d)
        else:
            # hardware loop re-running the full kernel body, for benchmarking
            # steady-state per-iteration execution time on device
            with tc.For_i(0, niter):
                body(nc, tc, x_d, wq_d, wk_d, wv_d, o_d)
    nc.compile()
    try:
        build_program.last_perfetto = tc._perfetto_entries
    except Exception:
        build_program.last_perfetto = None
    return nc


def _body(nc, tc, x_d, wq_d, wk_d, wv_d, o_d):
    from contextlib import ExitStack
    ctx = ExitStack()
    with ctx:
        consts = ctx.enter_context(tc.tile_pool(name="consts", bufs=1))
        big = ctx.enter_context(tc.tile_pool(name="big", bufs=1))
        xin = ctx.enter_context(tc.tile_pool(name="xin", bufs=3))
        ptile = ctx.enter_context(tc.tile_pool(name="ptile", bufs=4))

        # ---- constants ----
        ident = consts.tile([P, P], F32)
        make_identity(nc, ident[:])
        ident_r = consts.tile([P, P], F32R, tag="ident_r")
        nc.vector.tensor_copy(ident_r[:], ident[:])
        # mask01[d]: [128,512] f32r, keep (1.0) where t_local + off - s >= 0
        masks = []
        for d in range(4):
            mf = consts.tile([P, 512], F32, tag=f"maskf{d}")
            nc.gpsimd.memset(mf[:], 1.0)
            nc.gpsimd.affine_select(
                out=mf[:], in_=mf[:], compare_op=mybir.AluOpType.is_ge,
                fill=0.0, base=-d * P, channel_multiplier=-1,
                pattern=[[1, 512]],
            )
            m = consts.tile([P, 512], F32R, tag=f"mask{d}")
            nc.vector.tensor_copy(m[:], mf[:])
            masks.append(m)

        # ---- weights: [C,H] -> [128, 8, 64], fuse q|k, convert to f32r ----
        w_stage = {}
        for name, ap in (("q", wq_d), ("k", wk_d), ("v", wv_d)):
            ws = consts.tile([P, NCC, H], F32, tag=f"ws_{name}")
            nc.sync.dma_start(ws[:], ap.rearrange("(cc p) h -> p cc h", p=P))
            w_stage[name] = ws
        w_qk = consts.tile([P, NCC, P], F32R, tag="w_qk")
        nc.vector.tensor_copy(w_qk[:, :, 0:H], w_stage["q"][:])
        nc.vector.tensor_copy(w_qk[:, :, H:P], w_stage["k"][:])
        w_v = consts.tile([P, NCC, H], F32R, tag="w_v")
        nc.vector.tensor_copy(w_v[:], w_stage["v"][:])

        # ---- persistent big tensors ----
        xT = big.tile([P, NCC, T], F32R, tag="xT")        # x^T, c on partitions
        qkT = big.tile([P, T], F32R, tag="qkT")           # rows 0:64 qT, 64:128 kT
        kT = big.tile([H, T], F32R, tag="kT")             # kT at partitions 0:63
        vTs = big.tile([H, T], F32, tag="vT")             # v^T [h, s]
        v1 = big.tile([P, NT, H + 1], F32R, tag="v1")     # v natural + ones col
        o_sb = big.tile([P, NT, H], F32, tag="o")         # final output

        # ---- phase 1+2: load x, transpose, project ----
        with tc.tile_pool(name="psA", bufs=4, space="PSUM") as psA, \
             tc.tile_pool(name="psQK", bufs=2, space="PSUM") as psQK, \
             tc.tile_pool(name="psV", bufs=2, space="PSUM") as psV:
            x_r = x_d.rearrange("(n p) c -> p n c", p=P)  # [128, 16, 1024]
            for tg in range(4):                           # t-groups of 512
                xb = xin.tile([P, 4, C], F32R, tag="xb")
                nc.sync.dma_start(xb[:], x_r[:, tg * 4:(tg + 1) * 4, :])
                for cc in range(NCC):
                    ps = psA.tile([P, 512], F32R, tag="xp")
                    for i in range(4):
                        nc.tensor.transpose(
                            ps[:, i * P:(i + 1) * P],
                            xb[:, i, cc * P:(cc + 1) * P], ident_r[:])
                    # PSUM->SBUF copy converts to f32r; alternate ACT/DVE
                    dst = xT[:, cc, tg * 512:(tg + 1) * 512]
                    if cc % 2 == 0:
                        nc.scalar.copy(dst, ps[:])
                    else:
                        nc.vector.tensor_copy(dst, ps[:])
                # projections for this 512-wide t-slice (keeps PE warm too)
                pqk = psQK.tile([P, 512], F32, tag="qk")
                pv = psV.tile([H, 512], F32, tag="v")
                for cc in range(NCC):
                    rhs = xT[:, cc, tg * 512:(tg + 1) * 512]
                    nc.tensor.matmul(pqk[:], w_qk[:, cc, :], rhs,
                                     start=(cc == 0), stop=(cc == NCC - 1))
                for cc in range(NCC):
                    rhs = xT[:, cc, tg * 512:(tg + 1) * 512]
                    nc.tensor.matmul(pv[:], w_v[:, cc, :], rhs,
                                     start=(cc == 0), stop=(cc == NCC - 1))
                nc.vector.tensor_copy(qkT[:, tg * 512:(tg + 1) * 512], pqk[:])
                nc.vector.tensor_copy(vTs[:, tg * 512:(tg + 1) * 512], pv[:])

            # kT to partitions 0:63 (cross-partition: SBUF->SBUF DMA)
            nc.sync.dma_start(kT[:], qkT[H:P, :])

            # v natural: PE-transpose vT in 128-col chunks
            for ss in range(NT):
                pvt = psA.tile([P, H], F32, tag="xp")
                nc.tensor.transpose(pvt[:], vTs[:, ss * P:(ss + 1) * P],
                                    ident[0:H, 0:H])
                nc.vector.tensor_copy(v1[:, ss, 0:H], pvt[:])
            ones_f = consts.tile([P, NT], F32, tag="ones_f")
        nc.gpsimd.memset(ones_f[:], 1.0)
        nc.vector.tensor_copy(v1[:, :, H], ones_f[:])

        # ---- phase 3: attention ----
        psB = ctx.enter_context(tc.tile_pool(name="psB", bufs=2, space="PSUM"))
        psOT = ctx.enter_context(tc.tile_pool(name="psOT", bufs=4, space="PSUM"))
        ot = [psOT.tile([H + 1, 512], F32, tag="ot", name=f"ot{i}")
              for i in range(4)]

        for j in range(NT):                  # key block (s = j*128 ...)
            for u in range(j // 8, 2):       # 1024-wide t tiles
                halves = [h for h in range(2)
                          if (u * 1024 + (h + 1) * 512) > j * P]
                st = psB.tile([P, 1024], F32, tag="st")
                for h in halves:
                    nc.tensor.matmul(
                        st[:, h * 512:(h + 1) * 512],
                        kT[:, j * P:(j + 1) * P],
                        qkT[0:H, u * 1024 + h * 512: u * 1024 + (h + 1) * 512],
                        start=True, stop=True)
                pt = ptile.tile([P, 1024], F32R, tag="pt")
                if len(halves) == 2:
                    nc.scalar.activation(pt[:], st[:],
                                         mybir.ActivationFunctionType.Exp,
                                         scale=SCALE)
                else:
                    h = halves[0]
                    nc.scalar.activation(pt[:, h * 512:(h + 1) * 512],
                                         st[:, h * 512:(h + 1) * 512],
                                         mybir.ActivationFunctionType.Exp,
                                         scale=SCALE)
                if u == j // 8:              # diagonal tile: causal mask
                    hd = (j % 8) // 4
                    sl = slice(hd * 512, (hd + 1) * 512)
                    nc.vector.tensor_mul(out=pt[:, sl], in0=pt[:, sl],
                                         in1=masks[j % 4][:])
                for h in halves:
                    tcn = u * 2 + h
                    nc.tensor.matmul(
                        ot[tcn][:], v1[:, j, :],
                        pt[:, h * 512:(h + 1) * 512],
                        start=(j == 0), stop=(j == 4 * tcn + 3))

        # ---- phase 4: normalize + transpose back ----
        otsb_pool = ctx.enter_context(tc.tile_pool(name="otsb", bufs=2))
        rec_pool = ctx.enter_context(tc.tile_pool(name="rec", bufs=4))
        for tcn in range(4):
            osb = otsb_pool.tile([H + 1, 512], F32, tag="otsb")
            nc.vector.tensor_copy(osb[:], ot[tcn][:])
            for q in range(4):
                po = psB.tile([P, H + 1], F32, tag="st")
                nc.tensor.transpose(po[:], osb[:, q * P:(q + 1) * P],
                                    ident[0:H + 1, 0:H + 1])
                rec = rec_pool.tile([P, 1], F32, tag="rec")
                nc.vector.reciprocal(rec[:], po[:, H:H + 1])
                nc.vector.tensor_scalar_mul(
                    o_sb[:, tcn * 4 + q, :], po[:, 0:H], rec[:])
        nc.sync.dma_start(o_d.rearrange("(n p) h -> p n h", p=P), o_sb[:])


def _body_v2(nc, tc, x_d, wq_d, wk_d, wv_d, o_d):
    """Column-streaming, software-pipelined schedule.

    For each 512-wide query group g: load x rows, transpose, project q|k|v,
    then run the attention column (key blocks j=4g+3..0, descending) and emit
    output rows. Engines execute their queues in emission order, so group
    g+1's load/transpose/projection units are emitted INTERLEAVED between
    column g's attention iterations to overlap PE work with ACT/DMA latency.
    Diagonal key blocks only compute the live 256..512-wide slice.
    """
    from contextlib import ExitStack
    TG = 512
    ctx = ExitStack()
    with ctx:
        consts = ctx.enter_context(tc.tile_pool(name="consts", bufs=1))
        big = ctx.enter_context(tc.tile_pool(name="big", bufs=1))
        xin = ctx.enter_context(tc.tile_pool(name="xin", bufs=2))
        xTp = ctx.enter_context(tc.tile_pool(name="xT", bufs=2))
        qkp = ctx.enter_context(tc.tile_pool(name="qk", bufs=2))
        vTp = ctx.enter_context(tc.tile_pool(name="vT", bufs=2))
        ptile = ctx.enter_context(tc.tile_pool(name="ptile", bufs=4))
        osbp = ctx.enter_context(tc.tile_pool(name="osb", bufs=2))
        recp = ctx.enter_context(tc.tile_pool(name="rec", bufs=4))
        psA = ctx.enter_context(tc.tile_pool(name="psA", bufs=2, space="PSUM"))
        psQK = ctx.enter_context(tc.tile_pool(name="psQK", bufs=1, space="PSUM"))
        psS = ctx.enter_context(tc.tile_pool(name="psS", bufs=2, space="PSUM"))
        psO = ctx.enter_context(tc.tile_pool(name="psO", bufs=2, space="PSUM"))

        # ---- constants ----
        ident = consts.tile([P, P], F32)
        make_identity(nc, ident[:])
        ident_r = consts.tile([P, P], F32R, tag="ident_r")
        nc.vector.tensor_copy(ident_r[:], ident[:])
        ident_h = consts.tile([P, P], BF16, tag="ident_h")
        nc.vector.tensor_copy(ident_h[:], ident[:])
        # mask256[p, u]: keep (1.0) where u >= 128 + p. Cols 128:256 form the
        # causal triangle of a 128-wide diagonal sub-block; cols 0:128 are
        # all-zero (dead zone ahead of the d=3 diagonal).
        m256f = consts.tile([P, 256], F32, tag="m256f")
        nc.gpsimd.memset(m256f[:], 1.0)
        nc.gpsimd.affine_select(
            out=m256f[:], in_=m256f[:], compare_op=mybir.AluOpType.is_ge,
            fill=0.0, base=-P, channel_multiplier=-1, pattern=[[1, 256]])
        mask256 = consts.tile([P, 256], BF16, tag="m256")
        nc.vector.tensor_copy(mask256[:], m256f[:])
        tri = mask256[:, 128:256]

        x_r0 = x_d.rearrange("(n p) c -> p n c", p=P)
        # group-0 x load goes on the DMA queue FIRST (critical path); the
        # strided weight loads follow and hide under it
        xb0 = xin.tile([P, 4, C], F32R, tag="xb", name="xb_g0")
        nc.sync.dma_start(xb0[:], x_r0[:, 0:4, :])

        # ---- weights: [C,H] -> [128, 8, 64], fuse q|k, convert to bf16 ----
        w_stage = {}
        for name, ap in (("q", wq_d), ("k", wk_d), ("v", wv_d)):
            ws = consts.tile([P, NCC, H], F32, tag=f"ws_{name}")
            nc.sync.dma_start(ws[:], ap.rearrange("(cc p) h -> p cc h", p=P))
            w_stage[name] = ws
        w_qk = consts.tile([P, NCC, P], BF16, tag="w_qk")
        nc.vector.tensor_copy(w_qk[:, :, 0:H], w_stage["q"][:])
        nc.vector.tensor_copy(w_qk[:, :, H:P], w_stage["k"][:])
        w_v = consts.tile([P, NCC, H], BF16, tag="w_v")
        nc.vector.tensor_copy(w_v[:], w_stage["v"][:])

        # ---- persistent ----
        kT = big.tile([H, T], BF16, tag="kT")
        v1 = big.tile([P, NT, H + 1], BF16, tag="v1")
        o_sb = big.tile([P, NT, H], F32, tag="o")
        ones_f = consts.tile([P, NT], F32, tag="ones_f")
        nc.gpsimd.memset(ones_f[:], 1.0)
        nc.vector.tensor_copy(v1[:, :, H], ones_f[:])

        x_r = x_d.rearrange("(n p) c -> p n c", p=P)
        o_r = o_d.rearrange("(n p) h -> p n h", p=P)

        qk_tiles = {}

        def group_stream(g, xb=None):
            """Yield emit-callbacks: one pipeline unit of group-g prep each."""
            xT = xTp.tile([P, NCC, TG], BF16, tag="xT", name=f"xT{g}")
            if xb is None:
                xb = xin.tile([P, 4, C], F32R, tag="xb", name=f"xb{g}")

                def load():
                    nc.sync.dma_start(xb[:], x_r[:, g * 4:(g + 1) * 4, :])
                yield load

            def transp(bi, hf):
                ps = psA.tile([P, 4, P], F32R, tag="tp", name=f"tp{g}_{bi}{hf}")
                for ci in range(4):
                    cc = hf * 4 + ci
                    nc.tensor.transpose(
                        ps[:, ci, :],
                        xb[:, bi, cc * P:(cc + 1) * P], ident_r[:])
                # PSUM->SBUF copy narrows to bf16; all on DVE so the ACT
                # queue stays clear for the latency-critical exp's
                dst = xT[:, hf * 4:(hf + 1) * 4, bi * P:(bi + 1) * P]
                nc.vector.tensor_copy(dst, ps[:])
            for bi in range(4):
                for hf in range(2):
                    yield (lambda bi=bi, hf=hf: transp(bi, hf))

            pqk = psQK.tile([P, TG], F32, tag="pj", name=f"pqk{g}")

            def proj_qk(h):
                for cc in range(4 * h, 4 * h + 4):
                    nc.tensor.matmul(pqk[:], w_qk[:, cc, :], xT[:, cc, :],
                                     start=(cc == 0), stop=(cc == NCC - 1))
            yield lambda: proj_qk(0)
            yield lambda: proj_qk(1)

            qkT_g = qkp.tile([P, TG], BF16, tag="qkT", name=f"qkT{g}")
            qk_tiles[g] = qkT_g

            def qk_out():
                nc.scalar.copy(qkT_g[0:H, :], pqk[0:H, :])
                nc.vector.tensor_copy(qkT_g[H:P, :], pqk[H:P, :])
                # kT rows sit at partitions 64:128; relocate to 0:63
                nc.sync.dma_start(kT[:, g * TG:(g + 1) * TG], qkT_g[H:P, :])
            yield qk_out

            pv = psQK.tile([H, TG], F32, tag="pj", name=f"pv{g}")

            def proj_v(h):
                for cc in range(4 * h, 4 * h + 4):
                    nc.tensor.matmul(pv[:], w_v[:, cc, :], xT[:, cc, :],
                                     start=(cc == 0), stop=(cc == NCC - 1))
            yield lambda: proj_v(0)
            yield lambda: proj_v(1)

            def v_out():
                vT_g = vTp.tile([H, TG], BF16, tag="vT", name=f"vT{g}")
                nc.vector.tensor_copy(vT_g[:], pv[:])
                for ss in range(4):
                    pvt = psA.tile([P, H], BF16, tag="tp", name=f"pvt{g}_{ss}")
                    nc.tensor.transpose(pvt[:], vT_g[:, ss * P:(ss + 1) * P],
                                        ident_h[0:H, 0:H])
                    nc.vector.tensor_copy(v1[:, g * 4 + ss, 0:H], pvt[:])
            yield v_out

        # group 0 prep runs un-interleaved (nothing to overlap with yet)
        for unit in group_stream(0, xb=xb0):
            unit()

        for g in range(4):
            units = list(group_stream(g + 1)) if g < 3 else []
            qkT_g = qk_tiles[g]
            jmax = 4 * g + 3
            nj = jmax + 1
            emitted = 0
            ot = psO.tile([H + 1, TG], F32, tag="ot", name=f"ot{g}")
            pend = None   # AV is emitted one j late so PE never waits on exp
            for i, j in enumerate(range(jmax, -1, -1)):
                d = j - 4 * g
                col0 = [0, 128, 256, 256][d] if d >= 0 else 0
                cols = slice(col0, TG)
                st = psS.tile([P, TG], F32, tag="st", name=f"st{g}_{j}")
                nc.tensor.matmul(st[:, cols], kT[:, j * P:(j + 1) * P],
                                 qkT_g[0:H, cols], start=True, stop=True)
                pt = ptile.tile([P, TG], BF16, tag="pt", name=f"pt{g}_{j}")
                nc.scalar.activation(pt[:, cols], st[:, cols],
                                     mybir.ActivationFunctionType.Exp,
                                     scale=SCALE)
                if d == 3:
                    nc.vector.tensor_mul(out=pt[:, 256:512],
                                         in0=pt[:, 256:512], in1=mask256[:])
                elif d >= 0:
                    sl = slice(d * P, (d + 1) * P)
                    nc.vector.tensor_mul(out=pt[:, sl], in0=pt[:, sl],
                                         in1=tri)
                if pend is not None:
                    pj, ppt, pcols = pend
                    nc.tensor.matmul(ot[:, pcols], v1[:, pj, :], ppt[:, pcols],
                                     start=(pj == jmax), stop=False)
                pend = (j, pt, cols)
                # interleave next group's prep units across this column
                want = (i + 1) * len(units) // nj
                while emitted < want:
                    units[emitted]()
                    emitted += 1
            pj, ppt, pcols = pend
            nc.tensor.matmul(ot[:, pcols], v1[:, pj, :], ppt[:, pcols],
                             start=(pj == jmax), stop=True)

            # -- finalize column g: normalize, transpose back, store --
            osb = osbp.tile([H + 1, TG], F32, tag="osb", name=f"osb{g}")
            nc.vector.tensor_copy(osb[:], ot[:])
            for qq in range(4):
                po = psA.tile([P, H + 1], F32, tag="po", bufs=1,
                              name=f"po{g}_{qq}")
                nc.tensor.transpose(po[:], osb[:, qq * P:(qq + 1) * P],
                                    ident[0:H + 1, 0:H + 1])
                rec = recp.tile([P, 1], F32, tag="rec", name=f"rec{g}_{qq}")
                nc.vector.reciprocal(rec[:], po[:, H:H + 1])
                nc.vector.tensor_scalar_mul(
                    o_sb[:, g * 4 + qq, :], po[:, 0:H], rec[:])
            nc.sync.dma_start(o_r[:, g * 4:(g + 1) * 4, :],
                              o_sb[:, g * 4:(g + 1) * 4, :])


def _body_v4(nc, tc, x_d, wq_d, wk_d, wv_d, o_d):
    """v4: XBAR DMA-transpose datapath, bf16 matmuls, minimal PE instructions.

    x rows are loaded f32, narrowed to bf16 on DVE, and transposed by the
    DMA XBAR (one dma_start_transpose per 128-row t-block) straight into the
    [c-part, cc, t] layout projections need — eliminating all 128 PE
    transposes and their PSUM round-trips. q/k/v, scores and AV all run in
    bf16 (PSUM accumulates f32). Column-streaming + software pipelining as
    before: group g+1 prep interleaves with column g attention; AV lags one
    iteration so PE never waits on the exp.
    """
    from contextlib import ExitStack
    TG = 512
    ctx = ExitStack()
    with ctx:
        consts = ctx.enter_context(tc.tile_pool(name="consts", bufs=1))
        big = ctx.enter_context(tc.tile_pool(name="big", bufs=1))
        xin = ctx.enter_context(tc.tile_pool(name="xin", bufs=2))
        xhp = ctx.enter_context(tc.tile_pool(name="xh", bufs=2))
        xTp = ctx.enter_context(tc.tile_pool(name="xT", bufs=2))
        qkp = ctx.enter_context(tc.tile_pool(name="qk", bufs=2))
        vTp = ctx.enter_context(tc.tile_pool(name="vT", bufs=2))
        ptile = ctx.enter_context(tc.tile_pool(name="ptile", bufs=4))
        osbp = ctx.enter_context(tc.tile_pool(name="osb", bufs=2))
        recp = ctx.enter_context(tc.tile_pool(name="rec", bufs=4))
        psJ = ctx.enter_context(tc.tile_pool(name="psJ", bufs=2, space="PSUM"))
        psS = ctx.enter_context(tc.tile_pool(name="psS", bufs=4, space="PSUM"))
        psO = ctx.enter_context(tc.tile_pool(name="psO", bufs=2, space="PSUM"))

        x_r = x_d.rearrange("(n p) c -> p n c", p=P)
        o_r = o_d.rearrange("(n p) h -> p n h", p=P)

        # group-0 x load enters the SP DMA queue before anything else
        xb0 = xin.tile([P, 4, C], F32R, tag="xb", name="xb_g0")
        nc.sync.dma_start(xb0[:], x_r[:, 0:4, :])

        # ---- constants ----
        ident = consts.tile([P, P], F32)
        make_identity(nc, ident[:])
        # mask256[p, u]: keep (1.0) where u >= 128 + p (causal triangle with a
        # 128-wide all-zero dead zone ahead of it, for the d=3 diagonal)
        m256f = consts.tile([P, 256], F32, tag="m256f")
        nc.gpsimd.memset(m256f[:], 1.0)
        nc.gpsimd.affine_select(
            out=m256f[:], in_=m256f[:], compare_op=mybir.AluOpType.is_ge,
            fill=0.0, base=-P, channel_multiplier=-1, pattern=[[1, 256]])
        mask256 = consts.tile([P, 256], BF16, tag="m256")
        nc.vector.tensor_copy(mask256[:], m256f[:])
        tri = mask256[:, 128:256]

        # ---- weights: [C,H] -> [128, 8, 64] (c = cc*128 + p), bf16 ----
        w_stage = {}
        for name, ap in (("q", wq_d), ("k", wk_d), ("v", wv_d)):
            ws = consts.tile([P, NCC, H], F32, tag=f"ws_{name}")
            nc.sync.dma_start(ws[:], ap.rearrange("(cc p) h -> p cc h", p=P))
            w_stage[name] = ws
        w_qk = consts.tile([P, NCC, P], BF16, tag="w_qk")
        nc.vector.tensor_copy(w_qk[:, :, 0:H], w_stage["q"][:])
        nc.vector.tensor_copy(w_qk[:, :, H:P], w_stage["k"][:])
        w_v = consts.tile([P, NCC, H], BF16, tag="w_v")
        nc.vector.tensor_copy(w_v[:], w_stage["v"][:])

        # ---- persistent ----
        kT = big.tile([H, T], BF16, tag="kT")
        v1 = big.tile([P, NT, H + 1], BF16, tag="v1")
        o_sb = big.tile([P, NT, H], F32, tag="o")
        ones_f = consts.tile([P, NT], F32, tag="ones_f")
        nc.gpsimd.memset(ones_f[:], 1.0)
        nc.vector.tensor_copy(v1[:, :, H], ones_f[:])

        qk_tiles = {}

        def group_stream(g, xb=None):
            """Yield emit-callbacks: one pipeline unit of group-g prep each."""
            # xT4[c_lo, bi, cc, t_lo] = x^T[cc*128+c_lo, g*512+bi*128+t_lo]
            xT = xTp.tile([P, 4, NCC, P], BF16, tag="xT", name=f"xT{g}")
            xh = xhp.tile([P, 4, C], BF16, tag="xh", name=f"xh{g}")
            if xb is None:
                xb = xin.tile([P, 4, C], F32R, tag="xb", name=f"xb{g}")

                def load():
                    nc.sync.dma_start(xb[:], x_r[:, g * 4:(g + 1) * 4, :])
                yield load

            def conv(bi):
                nc.vector.tensor_copy(xh[:, bi, :], xb[:, bi, :])
            for bi in range(4):
                yield (lambda bi=bi: conv(bi))

            def xbar():
                # one XBAR transpose for the whole 512-row group (ACT queue,
                # emitted after all four converts so its wait is satisfied)
                nc.scalar.dma_start(xT[:], xh[:], transpose=True)
            yield xbar

            pqk = psJ.tile([P, TG], F32, tag="pj", name=f"pqk{g}")
            pv = psJ.tile([H, TG], F32, tag="pj", name=f"pv{g}")

            def proj(h):
                # interleave the qk and v chains so each LDWEIGHTS overlaps
                # the other chain's streaming matmul
                for cc in range(4 * h, 4 * h + 4):
                    nc.tensor.matmul(pqk[:], w_qk[:, cc, :], xT[:, :, cc, :],
                                     start=(cc == 0), stop=(cc == NCC - 1))
                    nc.tensor.matmul(pv[:], w_v[:, cc, :], xT[:, :, cc, :],
                                     start=(cc == 0), stop=(cc == NCC - 1))
            yield lambda: proj(0)
            yield lambda: proj(1)

            qkT_g = qkp.tile([P, TG], BF16, tag="qkT", name=f"qkT{g}")
            qk_tiles[g] = qkT_g

            def qk_out():
                nc.scalar.copy(qkT_g[0:H, :], pqk[0:H, :])
                nc.vector.tensor_copy(qkT_g[H:P, :], pqk[H:P, :])
                # kT rows sit at partitions 64:128; relocate to 0:63
                nc.sync.dma_start(kT[:, g * TG:(g + 1) * TG], qkT_g[H:P, :])
            yield qk_out

            def v_out():
                vT_g = vTp.tile([H, TG], BF16, tag="vT", name=f"vT{g}")
                nc.vector.tensor_copy(vT_g[:], pv[:])
                # XBAR-transpose vT to natural v rows; the XBAR needs a
                # contiguous destination, so land in vn then copy into v1
                vn = vTp.tile([P, 4, H], BF16, tag="vn", name=f"vn{g}")
                nc.scalar.dma_start(vn[:], vT_g[:], transpose=True)
                nc.vector.tensor_copy(v1[:, g * 4:(g + 1) * 4, 0:H], vn[:])
            yield v_out

        # group 0 prep runs un-interleaved (nothing to overlap with yet)
        for unit in group_stream(0, xb=xb0):
            unit()

        for g in range(4):
            units = list(group_stream(g + 1)) if g < 3 else []
            qkT_g = qk_tiles[g]
            jmax = 4 * g + 3
            nj = jmax + 1
            emitted = 0
            ot = psO.tile([H + 1, TG], F32, tag="ot", name=f"ot{g}")
            pend = None   # AV is emitted one j late so PE never waits on exp
            for i, j in enumerate(range(jmax, -1, -1)):
                d = j - 4 * g
                col0 = [0, 128, 256, 256][d] if d >= 0 else 0
                cols = slice(col0, TG)
                st = psS.tile([P, TG], F32, tag="st", name=f"st{g}_{j}")
                nc.tensor.matmul(st[:, cols], kT[:, j * P:(j + 1) * P],
                                 qkT_g[0:H, cols], start=True, stop=True)
                pt = ptile.tile([P, TG], BF16, tag="pt", name=f"pt{g}_{j}")
                nc.scalar.activation(pt[:, cols], st[:, cols],
                                     mybir.ActivationFunctionType.Exp,
                                     scale=SCALE)
                if d == 3:
                    nc.vector.tensor_mul(out=pt[:, 256:512],
                                         in0=pt[:, 256:512], in1=mask256[:])
                elif d >= 0:
                    sl = slice(d * P, (d + 1) * P)
                    nc.vector.tensor_mul(out=pt[:, sl], in0=pt[:, sl],
                                         in1=tri)
                if pend is not None:
                    pj_, ppt, pcols = pend
                    nc.tensor.matmul(ot[:, pcols], v1[:, pj_, :],
                                     ppt[:, pcols],
                                     start=(pj_ == jmax), stop=False)
                pend = (j, pt, cols)
                # interleave next group's prep units across this column
                want = (i + 1) * len(units) // nj
                while emitted < want:
                    units[emitted]()
                    emitted += 1
            pj_, ppt, pcols = pend
            nc.tensor.matmul(ot[:, pcols], v1[:, pj_, :], ppt[:, pcols],
                             start=(pj_ == jmax), stop=True)

            # -- finalize column g: XBAR-transpose OT back to natural rows,
            #    then normalize by the ones-row denominator (col 64) --
            osb = osbp.tile([80, TG], BF16, tag="osb", name=f"osb{g}")
            nc.vector.tensor_copy(osb[0:H + 1, :], ot[:])
            on = osbp.tile([P, 4, 80], BF16, tag="on", name=f"on{g}")
            nc.scalar.dma_start(on[:], osb[:], transpose=True)
            rec = recp.tile([P, 4], F32, tag="rec", name=f"rec{g}")
            nc.vector.reciprocal(rec[:], on[:, :, H])
            for qq in range(4):
                nc.vector.tensor_scalar_mul(
                    o_sb[:, g * 4 + qq, :], on[:, qq, 0:H],
                    rec[:, qq:qq + 1])
        nc.sync.dma_start(o_r[:], o_sb[:])


def kernel(x, Wq, Wk, Wv):
    key = "prog"
    if key not in _CACHE:
        _CACHE[key] = build_program()
    nc = _CACHE[key]
    B = x.shape[0]
    in_maps = [{"x": np.ascontiguousarray(x[b], dtype=np.float32),
                "Wq": np.asarray(Wq, dtype=np.float32),
                "Wk": np.asarray(Wk, dtype=np.float32),
                "Wv": np.asarray(Wv, dtype=np.float32)} for b in range(B)]
    res = bass_utils.run_bass_kernel_spmd(nc, in_maps, list(range(B)))
    return np.stack([res.results[b]["out"] for b in range(B)], axis=0)


def run_traced(x, Wq, Wk, Wv):
    """Same as kernel() but with NTFF tracing; returns (out, BassKernelResults)."""
    nc = build_program()
    B = x.shape[0]
    in_maps = [{"x": np.ascontiguousarray(x[b], dtype=np.float32),
                "Wq": np.asarray(Wq, dtype=np.float32),
                "Wk": np.asarray(Wk, dtype=np.float32),
                "Wv": np.asarray(Wv, dtype=np.float32)} for b in range(B)]
    res = bass_utils.run_bass_kernel_spmd(nc, in_maps, list(range(B)),
                                          trace=True)
    out = np.stack([res.results[b]["out"] for b in range(B)], axis=0)
    return out, res

